# revision 1
# baseline (speedup 1.0000x reference)
"""Trainium2 Bass kernel for CustomSelfAttentionWithBias (B=2, T=2048, C=1024, H=16).

Computes y = proj(softmax(mask(QK^T/sqrt(hd) + emphasis_col0)) @ V) where
qkv = x @ W_attn, with a causal bool mask and +1.0 emphasis on score column 0.

Sharding: 8 cores; core c handles batch b = c//4 and heads 4*(c%4) .. +4
(data parallel on B, tensor parallel on heads; c_proj row-sharded so each
core emits a partial y[b] that the host sums).

Dataflow per core (everything bf16 into the PE, fp32 PSUM):
  - host pre-transposes x[b] -> xT [C, T] and pre-slices/casts weights (bf16),
    pre-scales Wq by 1/sqrt(hd).
  - Q^T,K^T [64,T] per head and V [T,64] per head come straight out of
    matmuls against xT (no on-chip transposes anywhere).
  - scores are computed transposed: S^T[k_chunk 128, q 512] = K^T.T @ Q^T,
    the +1.0 emphasis for k==0 is added by a tiny rank-1 accumulate matmul.
  - exp on ScalarE (PSUM -> SBUF bf16), causal masking by multiplying the
    4 diagonal chunks with a precomputed 0/1 slab.
  - PV with lhsT = [V | ones]: one accumulation produces O^T[64, q] AND the
    softmax denominator row; normalization happens in the PSUM->SBUF copy
    (multiply by DMA-broadcast reciprocal of the denominator).
  - proj: y[t 128, c 512] accumulated over the 2 head-pair chunks, copied to
    fp16 and DMA'd out; host sums the 4 partials per batch in fp32.
"""

import numpy as np
import ml_dtypes

B, T, C = 2, 2048, 1024
H, HD = 16, 64
NH = 4            # heads per core
N_CORES = 8
QB = 512          # query block (columns of S^T per matmul)
KC = 128          # key chunk (partition dim of S^T)
G = 2             # key chunks per exp group
N_QB = T // QB    # 4
N_KC = T // KC    # 16
CCH = C // 128    # 8 contraction chunks for the projections
EMPHASIS = 1.0

_COMPILED = {}


def _build(causal: bool = True):
    import concourse.bass as bass
    import concourse.tile as tile
    import concourse.mybir as mybir
    from concourse import bacc

    f32 = mybir.dt.float32
    f16 = mybir.dt.float16
    bf16 = mybir.dt.bfloat16
    EXP = mybir.ActivationFunctionType.Exp

    nc = bacc.Bacc("TRN2", target_bir_lowering=False, debug=False)

    xT = nc.dram_tensor("xT", [C, T], bf16, kind="ExternalInput").ap()
    wq = nc.dram_tensor("wq", [C, NH * HD], bf16, kind="ExternalInput").ap()
    wk = nc.dram_tensor("wk", [C, NH * HD], bf16, kind="ExternalInput").ap()
    wv = nc.dram_tensor("wv", [C, NH * HD], bf16, kind="ExternalInput").ap()
    wp = nc.dram_tensor("wp", [NH * HD, C], bf16, kind="ExternalInput").ap()
    mk = nc.dram_tensor("mk", [128, 896], bf16, kind="ExternalInput").ap()
    y = nc.dram_tensor("y", [T, C], f16, kind="ExternalOutput").ap()

    with tile.TileContext(nc) as tc:
        _body(nc, tc, bass, mybir, xT, wq, wk, wv, wp, mk, y, causal,
              f32, f16, bf16, EXP)
    nc.compile()
    return nc


def _body(nc, tc, bass, mybir, xT, wq, wk, wv, wp, mk, y, causal,
          f32, f16, bf16, EXP):
    from contextlib import ExitStack

    ctx = ExitStack()
    singles = ctx.enter_context(tc.tile_pool(name="singles", bufs=1))
    # PSUM pools: st (2 banks x 3) + po (1 bank x 2) + py shares st slots
    ps_st = ctx.enter_context(tc.tile_pool(name="ps_st", bufs=2, space="PSUM"))
    ps_po = ctx.enter_context(tc.tile_pool(name="ps_po", bufs=4, space="PSUM"))
    pt_pool = ctx.enter_context(tc.tile_pool(name="pt_pool", bufs=4))
    nrm_pool = ctx.enter_context(tc.tile_pool(name="nrm_pool", bufs=3))
    y_pool = ctx.enter_context(tc.tile_pool(name="y_pool", bufs=3))

    # ---- resident inputs -------------------------------------------------
    xT_sb = singles.tile([128, CCH, T], bf16)
    nc.sync.dma_start(out=xT_sb, in_=xT.rearrange("(c p) t -> p c t", p=128))
    wq_sb = singles.tile([128, CCH, NH * HD], bf16)
    nc.sync.dma_start(out=wq_sb, in_=wq.rearrange("(c p) n -> p c n", p=128))
    wk_sb = singles.tile([128, CCH, NH * HD], bf16)
    nc.sync.dma_start(out=wk_sb, in_=wk.rearrange("(c p) n -> p c n", p=128))
    wv_sb = singles.tile([128, CCH, NH * HD], bf16)
    nc.sync.dma_start(out=wv_sb, in_=wv.rearrange("(c p) n -> p c n", p=128))
    wp_sb = singles.tile([128, 2, C], bf16)
    nc.sync.dma_start(out=wp_sb, in_=wp.rearrange("(j p) n -> p j n", p=128))
    mk_sb = singles.tile([128, 896], bf16)
    nc.sync.dma_start(out=mk_sb, in_=mk)

    # ---- qkv generation --------------------------------------------------
    # Q^T / K^T per head pair: [128 = 2 heads x 64, T]
    qt_sb = [singles.tile([128, T], bf16, name=f"qt{p}") for p in range(2)]
    kt_sb = [singles.tile([128, T], bf16, name=f"kt{p}") for p in range(2)]
    for pr in range(2):
        for dst_sb, w_sb in ((qt_sb[pr], wq_sb), (kt_sb[pr], wk_sb)):
            for nb in range(N_QB):
                pg = ps_st.tile([128, QB], f32, tag="st")
                for cc in range(CCH):
                    nc.tensor.matmul(
                        pg,
                        w_sb[:, cc, pr * 128:(pr + 1) * 128],
                        xT_sb[:, cc, nb * QB:(nb + 1) * QB],
                        start=(cc == 0), stop=(cc == CCH - 1),
                    )
                nc.vector.tensor_copy(dst_sb[:, nb * QB:(nb + 1) * QB], pg)

    # V | ones, keyed by key-chunk: [128 k, chunk, head, 65]
    v_sb = singles.tile([128, N_KC, NH, HD + 1], bf16)
    nc.vector.memset(v_sb[:, :, :, HD:HD + 1], 1.0)
    for kc in range(N_KC):
        pg = ps_st.tile([128, NH * HD], f32, tag="st")
        for cc in range(CCH):
            nc.tensor.matmul(
                pg,
                xT_sb[:, cc, kc * 128:(kc + 1) * 128],
                wv_sb[:, cc, :],
                start=(cc == 0), stop=(cc == CCH - 1),
            )
        nc.vector.tensor_copy(v_sb[:, kc, :, 0:HD], pg)
    # fold the column-0 emphasis into V|ones row for key 0: P gets exp(s+1)
    import math
    nc.scalar.mul(v_sb[0:1, 0, :, :], v_sb[0:1, 0, :, :], float(math.exp(EMPHASIS)))

    # O^T per head pair: [128 = 2 heads x 64, T]
    ot_sb = [singles.tile([128, T], bf16, name=f"ot{p}") for p in range(2)]

    # ---- attention + projection (proj trails attention by one block) ----
    def proj(qb):
        for tci in range(4):
            tc_i = 4 * qb + tci
            ysb = y_pool.tile([128, C], f16, tag="ysb")
            for ch in range(2):
                py = ps_st.tile([128, QB], f32, tag="st")
                for pr2 in range(2):
                    nc.tensor.matmul(
                        py,
                        ot_sb[pr2][:, tc_i * 128:(tc_i + 1) * 128],
                        wp_sb[:, pr2, ch * QB:(ch + 1) * QB],
                        start=(pr2 == 0), stop=(pr2 == 1),
                    )
                nc.vector.tensor_copy(ysb[:, ch * QB:(ch + 1) * QB], py)
            nc.sync.dma_start(
                out=y[tc_i * 128:(tc_i + 1) * 128, :], in_=ysb)

    # One PV group is kept pending so the PE stream interleaves
    # QK(g+1) between QK(g) and PV(g): exp/mask latency is hidden and the
    # PE never goes idle long enough for HAM to re-throttle. Each group is
    # one key chunk for BOTH heads of a pair: the two QK^T matmuls have
    # K=64 and run concurrently in disjoint PE row groups (rows 0-63 /
    # 64-127, auto tile_position from the operands' base partition).
    pending = []

    def norm(h, qb, po):
        # recip(den) broadcast, fused into the O^T PSUM->SBUF copy
        pr, s = h // 2, h % 2
        den = nrm_pool.tile([HD + 1, QB], f32, tag="den")
        nc.vector.tensor_copy(den[HD:HD + 1, :], po[HD:HD + 1, :])
        bde = nrm_pool.tile([HD, QB], f32, tag="bde")
        nc.sync.dma_start(
            out=bde,
            in_=den[HD:HD + 1, :].unsqueeze(1).broadcast_to([1, HD, QB]))
        rec = nrm_pool.tile([HD, QB], f32, tag="rec")
        nc.vector.reciprocal_approx_fast(out=rec, in_=bde)
        if s == 0:
            nc.vector.tensor_mul(
                ot_sb[pr][0:HD, qb * QB:(qb + 1) * QB], po[0:HD, :], rec)
        else:
            osh = nrm_pool.tile([HD, QB], bf16, tag="osh")
            nc.vector.tensor_mul(osh, po[0:HD, :], rec)
            nc.sync.dma_start(
                out=ot_sb[pr][HD:128, qb * QB:(qb + 1) * QB], in_=osh)

    def emit_pv(rec):
        pr, qb, kc, pt, po0, po1, nk = rec
        nc.tensor.matmul(po0, v_sb[:, kc, 2 * pr, :], pt[:, 0:QB],
                         start=(kc == 0), stop=(kc == nk - 1))
        nc.tensor.matmul(po1, v_sb[:, kc, 2 * pr + 1, :], pt[:, QB:2 * QB],
                         start=(kc == 0), stop=(kc == nk - 1))
        if kc == nk - 1:
            # s=1 (with its extra shift DMA) first so the block's last norm
            # chain, which gates the trailing projection, is the short one
            norm(2 * pr + 1, qb, po1)
            norm(2 * pr, qb, po0)

    for qb in range(N_QB):
        for pr in range(2):
            nk = 4 * (qb + 1) if causal else N_KC
            po0 = ps_po.tile([HD + 1, QB], f32, tag="po", name="po0")
            po1 = ps_po.tile([HD + 1, QB], f32, tag="po", name="po1")
            for kc in range(nk):
                st = ps_st.tile([128, 2 * QB], f32, tag="st")
                for s in range(2):
                    r0, r1 = s * HD, (s + 1) * HD
                    nc.tensor.matmul(
                        st[:, s * QB:(s + 1) * QB],
                        kt_sb[pr][r0:r1, kc * 128:(kc + 1) * 128],
                        qt_sb[pr][r0:r1, qb * QB:(qb + 1) * QB],
                        start=True, stop=True,
                    )
                pt = pt_pool.tile([128, 2 * QB], bf16, tag="pt")
                nc.scalar.activation(out=pt, in_=st, func=EXP)
                r = kc - 4 * qb
                if causal and r >= 0:
                    m0 = 384 - 128 * r
                    for s in range(2):
                        nc.vector.tensor_mul(
                            pt[:, s * QB:(s + 1) * QB],
                            pt[:, s * QB:(s + 1) * QB],
                            mk_sb[:, m0:m0 + QB],
                        )
                while len(pending) >= 2:
                    emit_pv(pending.pop(0))
                pending.append((pr, qb, kc, pt, po0, po1, nk))

        if qb >= 1:
            while pending and pending[0][1] < qb:
                emit_pv(pending.pop(0))
            proj(qb - 1)
    while pending:
        emit_pv(pending.pop(0))
    proj(N_QB - 1)

    ctx.close()


def _prep_inputs(x, W_attn, W_proj, attn_mask):
    """Host-side shard + layout prep. Returns (in_maps, causal)."""
    bf = ml_dtypes.bfloat16
    causal = bool(np.array_equal(
        np.asarray(attn_mask),
        np.tril(np.ones((T, T), dtype=bool))))

    x = np.asarray(x, dtype=np.float32)
    Wa = np.asarray(W_attn, dtype=np.float32)
    Wp = np.asarray(W_proj, dtype=np.float32)

    scale = 1.0 / np.sqrt(np.float32(HD))
    xT_b = [np.ascontiguousarray(x[b].T).astype(bf) for b in range(B)]

    # causal diagonal-mask slab: mk[i, m] = 1.0 if i <= m - 384 else 0
    i = np.arange(128)[:, None]
    m = np.arange(896)[None, :]
    mk = (i <= (m - 384)).astype(bf)

    in_maps = []
    for core in range(N_CORES):
        b, h0 = core // 4, (core % 4) * NH
        hsl = slice(h0 * HD, (h0 + NH) * HD)
        wq_c = np.ascontiguousarray(Wa[:, hsl] * scale).astype(bf)
        wk_c = np.ascontiguousarray(Wa[:, C + h0 * HD: C + (h0 + NH) * HD]).astype(bf)
        wv_c = np.ascontiguousarray(Wa[:, 2 * C + h0 * HD: 2 * C + (h0 + NH) * HD]).astype(bf)
        wp_c = np.ascontiguousarray(Wp[hsl, :]).astype(bf)
        in_maps.append({
            "xT": xT_b[b], "wq": wq_c, "wk": wk_c, "wv": wv_c,
            "wp": wp_c, "mk": mk,
        })
    return in_maps, causal


def kernel(x, W_attn, W_proj, attn_mask, _trace=False):
    from concourse import bass_utils

    in_maps, causal = _prep_inputs(x, W_attn, W_proj, attn_mask)
    key = ("causal" if causal else "dense")
    if key not in _COMPILED:
        _COMPILED[key] = _build(causal)
    nc = _COMPILED[key]

    res = bass_utils.run_bass_kernel_spmd(
        nc, in_maps, core_ids=list(range(N_CORES)), trace=_trace)

    y = np.zeros((B, T, C), dtype=np.float32)
    for core in range(N_CORES):
        y[core // 4] += res.results[core]["y"].astype(np.float32)
    if _trace:
        kernel._last_results = res
    return y

